# revision 1
# baseline (speedup 1.0000x reference)
"""Bass/Trainium2 kernel for nn_EnhancedPEFTWindowAttention.

Data-parallel over B_ (2048 windows*batch) across 8 NeuronCores:
256 windows = 12544 tokens per core. Weights / bias tables replicated.

Layout strategy (per core):
  - x is pre-transposed on host to channel-rows [128, 3, T] bf16 so every
    linear-layer matmul contracts over the partition dim with zero device
    transposes.
  - qkv LoRA-linear computed in channel-rows -> qkvT [128, 9, T] bf16
    (q/k/v each [384, T], head h at partition base 32*(h%4), k-tile h//4).
  - Attention per window pair: scores S^T[keys, (head, query)] via 24
    K=32 matmuls (w1 block at psum partition base 64), softmax as
    exp(S) * exp(bias+mask) with row sums from a ones-column appended to
    V, AV -> O token-rows, 3 PE transposes -> O^T channel-rows.
  - proj LoRA-linear in channel-rows -> outT [128, 3, T] f32, written
    transposed; host un-transposes.
"""

import sys

sys.path.insert(0, "/opt/trn_rl_repo")

import numpy as np
import ml_dtypes

import concourse.bacc as bacc
import concourse.tile as tile
from concourse import mybir
from concourse.bass_utils import run_bass_kernel_spmd

BF16 = ml_dtypes.bfloat16

WS = 7
N = 49
H = 12
D = 384
HD = 32
NW = 64
B_ = 2048
R = 16
SCALING = 32.0 / 16.0
SCALE = HD ** -0.5

NCORES = 8
WPC = B_ // NCORES            # windows per core = 256
TPC = WPC * N                 # tokens per core = 12544
WCHUNK = 8                    # windows per chunk
TCHUNK = WCHUNK * N           # 392 tokens per chunk
NCHUNK = WPC // WCHUNK        # 32 chunks
NPAIR = WCHUNK // 2           # 4 pairs per chunk

F32 = mybir.dt.float32
BF = mybir.dt.bfloat16

_COMPILED = {}


def _build(has_mask: bool):
    nc = bacc.Bacc("TRN2", target_bir_lowering=False, debug=False,
                   num_devices=NCORES)

    def din(name, shape, dt):
        return nc.dram_tensor(name, shape, dt, kind="ExternalInput").ap()

    xt_d = din("xt", [128, 3, TPC], BF)
    wqkvT_d = din("wqkvT", [128, 3, 3 * D], BF)
    wgT_d = din("wgT", [128, 3, 3 * D], BF)
    downT_d = din("downT", [128, 3, R], BF)
    upT_d = din("upT", [R, 3 * D], BF)
    pwT_d = din("pwT", [128, 3, D], BF)
    pgT_d = din("pgT", [128, 3, D], BF)
    pdownT_d = din("pdownT", [128, 3, R], BF)
    pupT_d = din("pupT", [R, D], BF)
    brow_d = din("brow", [1, 3 * D + D], BF)
    n_expb = 32 if has_mask else 1
    expb_d = din("expb", [n_expb, 113, 588], F32)
    ident_d = din("ident", [128, 128], BF)
    outT_d = nc.dram_tensor("outT", [128, 3, TPC], F32,
                            kind="ExternalOutput").ap()

    with tile.TileContext(nc) as tc:
        consts = tc.alloc_tile_pool(name="consts", bufs=1)
        xt_p = tc.alloc_tile_pool(name="xt", bufs=3)
        qkvT_p = tc.alloc_tile_pool(name="qkvT", bufs=2)
        sb_p = tc.alloc_tile_pool(name="sb", bufs=3)
        attn_p = tc.alloc_tile_pool(name="attn", bufs=2)
        oT_p = tc.alloc_tile_pool(name="oT", bufs=2)
        out_p = tc.alloc_tile_pool(name="out", bufs=2)
        ps_lin = tc.alloc_tile_pool(name="ps_lin", bufs=3, space="PSUM")
        ps_s_p = tc.alloc_tile_pool(name="ps_s", bufs=1, space="PSUM")
        ps_o_p = tc.alloc_tile_pool(name="ps_o", bufs=1, space="PSUM")
        ps_vt_p = tc.alloc_tile_pool(name="ps_vt", bufs=1, space="PSUM")

        # ---- resident constants ----
        wqkvT = consts.tile([128, 3, 3 * D], BF)
        nc.sync.dma_start(out=wqkvT, in_=wqkvT_d[:])
        wgT = consts.tile([128, 3, 3 * D], BF)
        nc.sync.dma_start(out=wgT, in_=wgT_d[:])
        downT = consts.tile([128, 3, R], BF)
        nc.sync.dma_start(out=downT, in_=downT_d[:])
        upT = consts.tile([R, 3 * D], BF)
        nc.sync.dma_start(out=upT, in_=upT_d[:])
        pwT = consts.tile([128, 3, D], BF)
        nc.sync.dma_start(out=pwT, in_=pwT_d[:])
        pgT = consts.tile([128, 3, D], BF)
        nc.sync.dma_start(out=pgT, in_=pgT_d[:])
        pdownT = consts.tile([128, 3, R], BF)
        nc.sync.dma_start(out=pdownT, in_=pdownT_d[:])
        pupT = consts.tile([R, D], BF)
        nc.sync.dma_start(out=pupT, in_=pupT_d[:])
        brow = consts.tile([1, 3 * D + D], BF)
        nc.sync.dma_start(out=brow, in_=brow_d[:])
        ones_row = consts.tile([1, 512], BF)
        nc.gpsimd.memset(ones_row[:], 1.0)
        ident = consts.tile([128, 128], BF)
        nc.sync.dma_start(out=ident, in_=ident_d[:])
        expb_c = None
        if not has_mask:
            expb_c = consts.tile([113, 2, 294], F32)
            nc.sync.dma_start(out=expb_c, in_=expb_d[0])

        # persistent attention tiles (manual double-buffer where needed):
        # one-time memsets of the pair-packing gap rows / ones columns
        ps_s = ps_s_p.tile([113, 2, 512], F32, tag="s")
        nc.vector.memset(ps_s[32:64, :, 0:294], 0.0)
        v_exts = []
        for i in range(2):
            ve = consts.tile([113, H, HD + 1], BF, name=f"v_ext{i}")
            nc.vector.memset(ve[:, :, HD:HD + 1], 1.0)
            v_exts.append(ve)
        o_sbs = []
        for i in range(2):
            ob = consts.tile([113, H, HD], BF, name=f"o_sb{i}")
            nc.vector.memset(ob[32:64, :, :], 0.0)
            o_sbs.append(ob)

        qkvT_tiles = {}
        oT_tiles = {}

        def emit_A(c):
            """qkv LoRA-linear for chunk c; yields after xd and each m-tile."""
            t0 = c * TCHUNK
            xt = xt_p.tile([128, 3, TCHUNK], BF, name="xt")
            nc.sync.dma_start(out=xt, in_=xt_d[:, :, t0:t0 + TCHUNK])
            qkvT = qkvT_p.tile([128, 9, TCHUNK], BF, name="qkvT")
            qkvT_tiles[c] = qkvT

            # xd^T = down @ x^T  [16, TCHUNK]
            ps_xd = ps_lin.tile([128, 512], F32, tag="lin", name="ps_xd")[0:R, 0:TCHUNK]
            for ki in range(3):
                nc.tensor.matmul(ps_xd, downT[:, ki, :], xt[:, ki, :],
                                 start=(ki == 0), stop=(ki == 2))
            xd = sb_p.tile([R, TCHUNK], BF, tag="xd", name="xd")
            nc.scalar.copy(xd[:], ps_xd[:])
            yield

            for mi in range(9):
                ps_g = ps_lin.tile([128, 512], F32, tag="lin", name="ps_g")[:, 0:TCHUNK]
                for ki in range(3):
                    nc.tensor.matmul(ps_g, wgT[:, ki, 128 * mi:128 * mi + 128],
                                     xt[:, ki, :],
                                     start=(ki == 0), stop=(ki == 2))
                g = sb_p.tile([128, TCHUNK], BF, tag="g", name="g")
                nc.scalar.activation(g[:], ps_g[:],
                                     mybir.ActivationFunctionType.Sigmoid)

                ps_l = ps_lin.tile([128, 512], F32, tag="lin", name="ps_l")[:, 0:TCHUNK]
                nc.tensor.matmul(ps_l, upT[:, 128 * mi:128 * mi + 128], xd[:],
                                 start=True, stop=True)

                ps_m = ps_lin.tile([128, 512], F32, tag="lin", name="ps_m")[:, 0:TCHUNK]
                for ki in range(3):
                    nc.tensor.matmul(ps_m, wqkvT[:, ki, 128 * mi:128 * mi + 128],
                                     xt[:, ki, :],
                                     start=(ki == 0), stop=False)
                # bias add as a K=1 matmul: ps_m += brow ⊗ ones
                nc.tensor.matmul(ps_m, brow[:, 128 * mi:128 * mi + 128],
                                 ones_row[:, 0:TCHUNK],
                                 start=False, stop=True)
                gl = sb_p.tile([128, TCHUNK], F32, tag="gl", name="gl")
                nc.vector.tensor_tensor(gl[:], g[:], ps_l[:],
                                        mybir.AluOpType.mult)
                nc.vector.tensor_tensor(qkvT[:, mi, :], ps_m[:], gl[:],
                                        mybir.AluOpType.add)
                yield

        def emit_B_pair(c, p):
            emit_B_front(c, p)
            emit_B_back(c, p)

        def emit_B_front(c, p):
            """attention for pair p of chunk c (through AV + normalize)."""
            qkvT = qkvT_tiles[c]
            if p == 0:
                oT_tiles[c] = oT_p.tile([128, 3, TCHUNK], BF, name="oT")
            oT = oT_tiles[c]
            pc0 = p * 2 * N
            if has_mask:
                expb = attn_p.tile([113, 2, 294], F32, tag="expb", name="expb")
                pm = (c * NPAIR + p) % 32
                nc.sync.dma_start(out=expb, in_=expb_d[pm])
            else:
                expb = expb_c

            # ---- hardware constraint driving the structure below:
            # concurrently-in-flight matmuls on different PE row groups
            # must write different PSUM banks. QK heads live at 4 row
            # bases (32*(h%4)); we run row groups {0,1} (banks 0,1),
            # then the full-K V-transposes (which conflict with every
            # row group, acting as a PE barrier), then groups {2,3}
            # reusing banks 0,1. AV w0/w1 use row groups {0,1}/{2,3}
            # and are split across banks by w.
            scol = lambda h: 49 * (3 * ((h % 4) // 2) + h // 4)

            # QK batch 1: heads with h%4 in {0,1}
            qk_b1 = []
            for w in range(2):
                wc0 = pc0 + w * N
                for h in range(12):
                    if h % 4 >= 2:
                        continue
                    mm = nc.tensor.matmul(
                        ps_s[64 * w:64 * w + N, h % 2,
                             scol(h):scol(h) + 49],
                        qkvT[32 * (h % 4):32 * (h % 4) + 32, 3 + h // 4,
                             wc0:wc0 + N],
                        qkvT[32 * (h % 4):32 * (h % 4) + 32, h // 4,
                             wc0:wc0 + N],
                        start=True, stop=True,
                        tile_position=(32 * (h % 4), 64 * w))
                    qk_b1.append(mm.ins)

            # V transposes (full-K: PE barrier between QK batches)
            v_ext = v_exts[p % 2]
            vt_insts = []
            ps_vt = ps_vt_p.tile([113, 3, 128], BF, tag="vt", name="ps_vt")
            for w in range(2):
                wc0 = pc0 + w * N
                for ki in range(3):
                    t = nc.tensor.transpose(
                        ps_vt[64 * w:64 * w + N, ki, :],
                        qkvT[:, 6 + ki, wc0:wc0 + N],
                        ident[:],
                        tile_position=(0, 64 * w))
                    for prev in qk_b1:
                        tile.add_dep_helper(t.ins, prev,
                                            reason="qk-b1 before vt")
                    vt_insts.append(t.ins)
            for w in range(2):
                nc.scalar.copy(
                    v_ext[64 * w:64 * w + N, :, 0:HD],
                    ps_vt[64 * w:64 * w + N, :, :].rearrange(
                        "p a (b c) -> p (a b) c", c=HD))

            # QK batch 2: heads with h%4 in {2,3}, after the barrier
            for w in range(2):
                wc0 = pc0 + w * N
                for h in range(12):
                    if h % 4 < 2:
                        continue
                    mm = nc.tensor.matmul(
                        ps_s[64 * w:64 * w + N, h % 2,
                             scol(h):scol(h) + 49],
                        qkvT[32 * (h % 4):32 * (h % 4) + 32, 3 + h // 4,
                             wc0:wc0 + N],
                        qkvT[32 * (h % 4):32 * (h % 4) + 32, h // 4,
                             wc0:wc0 + N],
                        start=True, stop=True,
                        tile_position=(32 * (h % 4), 64 * w))
                    for vt in vt_insts:
                        tile.add_dep_helper(mm.ins, vt,
                                            reason="vt before qk-b2")

            s_sb = attn_p.tile([113, 2, 294], F32, tag="ssb", name="s_sb")
            nc.vector.tensor_copy(s_sb[:], ps_s[:, :, 0:294])
            e_sb = attn_p.tile([113, 2, 294], F32, tag="e", name="e_sb")
            nc.scalar.activation(e_sb[:], s_sb[:],
                                 mybir.ActivationFunctionType.Exp)
            ep = attn_p.tile([113, 2, 294], BF, tag="ep", name="ep")
            nc.vector.tensor_tensor(ep[:], e_sb[:], expb[:],
                                    mybir.AluOpType.mult)

            # AV (+ rowsum in col 32 via ones column of v_ext);
            # psum bank = w so w0/w1 row-group sets never share a bank
            ps_o_t = ps_o_p.tile([113, 2, 512], F32, tag="o", name="ps_o")
            ps_o = [
                ps_o_t[64 * w:64 * w + N, w, 0:H * (HD + 1)].rearrange(
                    "p (h c) -> p h c", c=HD + 1)
                for w in range(2)
            ]
            for w in range(2):
                for h in range(12):
                    nc.tensor.matmul(
                        ps_o[w][:, h, :],
                        ep[64 * w:64 * w + N, h % 2,
                           scol(h):scol(h) + 49],
                        v_ext[64 * w:64 * w + N, h, :],
                        start=True, stop=True,
                        tile_position=(64 * w, 64 * w))

            r = attn_p.tile([113, H, 1], F32, tag="r", name="r")
            for w in range(2):
                nc.vector.reciprocal(r[64 * w:64 * w + N, :, :],
                                     ps_o[w][:, :, HD:HD + 1])
            o_sb = o_sbs[p % 2]
            for w in range(2):
                nc.vector.tensor_tensor(
                    o_sb[64 * w:64 * w + N, :, :],
                    ps_o[w][:, :, 0:HD],
                    r[64 * w:64 * w + N, :, :].to_broadcast([N, H, HD]),
                    mybir.AluOpType.mult)

        def emit_B_back(c, p):
            oT = oT_tiles[c]
            pc0 = p * 2 * N
            o_sb = o_sbs[p % 2]
            # O^T: transpose [113, 128] -> [128, 113], copy valid halves
            ps_ot = ps_vt_p.tile([128, 3, 128], BF, tag="vt", name="ps_ot")
            for ki in range(3):
                nc.tensor.transpose(ps_ot[:, ki, 0:113],
                                    o_sb[:, 4 * ki:4 * ki + 4, :],
                                    ident[0:113, 0:113])
            for w in range(2):
                nc.scalar.copy(
                    oT[:, :, pc0 + w * N:pc0 + (w + 1) * N],
                    ps_ot[:, :, 64 * w:64 * w + N])

        def emit_C(c):
            """proj LoRA-linear for chunk c; yields after pxd and each m-tile."""
            t0 = c * TCHUNK
            oT = oT_tiles.pop(c)
            qkvT_tiles.pop(c, None)
            ps_pxd = ps_lin.tile([128, 512], F32, tag="lin", name="ps_pxd")[0:R, 0:TCHUNK]
            for ki in range(3):
                nc.tensor.matmul(ps_pxd, pdownT[:, ki, :], oT[:, ki, :],
                                 start=(ki == 0), stop=(ki == 2))
            pxd = sb_p.tile([R, TCHUNK], BF, tag="xd", name="pxd")
            nc.scalar.copy(pxd[:], ps_pxd[:])
            yield

            out_sb = out_p.tile([128, 3, TCHUNK], F32, name="out_sb")
            for mi in range(3):
                ps_g2 = ps_lin.tile([128, 512], F32, tag="lin", name="ps_g2")[:, 0:TCHUNK]
                for ki in range(3):
                    nc.tensor.matmul(ps_g2, pgT[:, ki, 128 * mi:128 * mi + 128],
                                     oT[:, ki, :],
                                     start=(ki == 0), stop=(ki == 2))
                g2 = sb_p.tile([128, TCHUNK], BF, tag="g", name="g2")
                nc.scalar.activation(g2[:], ps_g2[:],
                                     mybir.ActivationFunctionType.Sigmoid)

                ps_l2 = ps_lin.tile([128, 512], F32, tag="lin", name="ps_l2")[:, 0:TCHUNK]
                nc.tensor.matmul(ps_l2, pupT[:, 128 * mi:128 * mi + 128],
                                 pxd[:], start=True, stop=True)

                ps_m2 = ps_lin.tile([128, 512], F32, tag="lin", name="ps_m2")[:, 0:TCHUNK]
                for ki in range(3):
                    nc.tensor.matmul(ps_m2, pwT[:, ki, 128 * mi:128 * mi + 128],
                                     oT[:, ki, :],
                                     start=(ki == 0), stop=False)
                nc.tensor.matmul(ps_m2, brow[:, 3 * D + 128 * mi:3 * D + 128 * mi + 128],
                                 ones_row[:, 0:TCHUNK],
                                 start=False, stop=True)
                gl2 = sb_p.tile([128, TCHUNK], F32, tag="gl", name="gl2")
                nc.vector.tensor_tensor(gl2[:], g2[:], ps_l2[:],
                                        mybir.AluOpType.mult)
                nc.vector.tensor_tensor(out_sb[:, mi, :], ps_m2[:], gl2[:],
                                        mybir.AluOpType.add)
                yield

            nc.sync.dma_start(out=outT_d[:, :, t0:t0 + TCHUNK], in_=out_sb)


        # clean schedule: for each c, emit A(c) m-tiles interleaved with
        # B pairs of c-1, then C(c-1) m-tiles interleaved into A(c) tail.
        prev_b = None  # chunk index whose B/C still need emission
        for c in range(NCHUNK):
            a_gen = emit_A(c)
            plan = []
            if prev_b is not None:
                for p in range(NPAIR):
                    plan.append(("Bf", (prev_b, p)))
                    plan.append(("Bb", (prev_b, p)))
                plan.append(("C", prev_b))
            # a_gen yields 10 times (xd + 9 m-tiles)
            c_sub = None
            for step in range(10):
                try:
                    next(a_gen)
                except StopIteration:
                    break
                if step == 0:
                    continue
                if plan:
                    kind, arg = plan[0]
                    if kind == "Bf":
                        emit_B_front(*arg)
                        plan.pop(0)
                    elif kind == "Bb":
                        emit_B_back(*arg)
                        plan.pop(0)
                    else:
                        if c_sub is None:
                            c_sub = emit_C(arg)
                        try:
                            next(c_sub)
                        except StopIteration:
                            c_sub = None
                            plan.pop(0)
            # finish any leftover B/C of prev chunk
            while plan:
                kind, arg = plan.pop(0)
                if kind == "Bf":
                    emit_B_front(*arg)
                elif kind == "Bb":
                    emit_B_back(*arg)
                else:
                    if c_sub is None:
                        c_sub = emit_C(arg)
                    for _ in c_sub:
                        pass
                    c_sub = None
            if c_sub is not None:
                for _ in c_sub:
                    pass
                c_sub = None
            prev_b = c
        # trailing B + C of the last chunk
        for p in range(NPAIR):
            emit_B_front(prev_b, p)
            emit_B_back(prev_b, p)
        for _ in emit_C(prev_b):
            pass

        for pool in reversed((consts, xt_p, qkvT_p, sb_p, attn_p, oT_p, out_p,
                              ps_lin, ps_s_p, ps_o_p, ps_vt_p)):
            pool.release()

    nc.compile()
    return nc


def _get_nc(has_mask: bool):
    if has_mask not in _COMPILED:
        _COMPILED[has_mask] = _build(has_mask)
    return _COMPILED[has_mask]


def _arr_lhsT(w_t, kparts):
    """[K, M] -> [128, K//128, M] partition-tiled lhsT layout."""
    K, M = w_t.shape
    return np.ascontiguousarray(
        w_t.reshape(kparts, 128, M).transpose(1, 0, 2))


def _prep_inputs(x, mask, qkv_w, qkv_b, qkv_down, qkv_up, qkv_gate, qkv_res,
                 proj_w, proj_b, proj_down, proj_up, proj_gate, proj_res,
                 bias_table, rel_index):
    x = np.asarray(x, np.float32)
    mask = np.asarray(mask, np.float32)
    has_mask = bool(np.any(mask))

    w_eff = (np.asarray(qkv_w, np.float32)
             + np.asarray(qkv_res, np.float32))        # [1152, 384]
    up_eff = np.asarray(qkv_up, np.float32) * SCALING  # [1152, 16]
    b_eff = np.asarray(qkv_b, np.float32).copy()
    # fold attention scale into the q-channel outputs
    w_eff[0:D] *= SCALE
    up_eff[0:D] *= SCALE
    b_eff[0:D] *= SCALE

    pw_eff = (np.asarray(proj_w, np.float32)
              + np.asarray(proj_res, np.float32))
    pup_eff = np.asarray(proj_up, np.float32) * SCALING

    common = {
        "wqkvT": _arr_lhsT(w_eff.T, 3).astype(BF16),
        "wgT": _arr_lhsT(np.asarray(qkv_gate, np.float32).T, 3).astype(BF16),
        "downT": _arr_lhsT(np.asarray(qkv_down, np.float32).T, 3).astype(BF16),
        "upT": np.ascontiguousarray(up_eff.T).astype(BF16),
        "pwT": _arr_lhsT(pw_eff.T, 3).astype(BF16),
        "pgT": _arr_lhsT(np.asarray(proj_gate, np.float32).T, 3).astype(BF16),
        "pdownT": _arr_lhsT(np.asarray(proj_down, np.float32).T, 3).astype(BF16),
        "pupT": np.ascontiguousarray(pup_eff.T).astype(BF16),
        "brow": np.concatenate(
            [b_eff, np.asarray(proj_b, np.float32)])[None, :].astype(BF16),
        "ident": np.eye(128, dtype=BF16),
    }

    # exp(relative-position bias + mask) in S^T layout [m, bank*294+scol(h)+n]
    bt = np.asarray(bias_table, np.float32)
    ri = np.asarray(rel_index).astype(np.int64)
    b_nmh = bt[ri]                                # [n, m, H]
    # head -> flat score column (bank = h%2, scol within bank)
    hperm = np.argsort(
        [(h % 2) * 6 + 3 * ((h % 4) // 2) + h // 4 for h in range(H)])
    # hperm[slot] = head occupying flat slot; eT cols laid out [slot, n]

    def _eT(lg):                                  # lg [n, m, H] -> [m, 588]
        e = np.exp(lg).transpose(1, 2, 0)         # [m, H, n]
        return e[:, hperm, :].reshape(N, 588)

    if has_mask:
        expb = np.zeros((32, 113, 588), np.float32)
        for pm in range(32):
            for w01 in range(2):
                lg = b_nmh + mask[2 * pm + w01][:, :, None]   # [n, m, H]
                expb[pm, 64 * w01:64 * w01 + N] = _eT(lg)
    else:
        expb = np.zeros((1, 113, 588), np.float32)
        eT = _eT(b_nmh)
        expb[0, 0:N] = eT
        expb[0, 64:64 + N] = eT
    common["expb"] = expb

    in_maps = []
    for core in range(NCORES):
        tok = np.ascontiguousarray(
            x[core * WPC:(core + 1) * WPC].reshape(TPC, D))
        xt = np.ascontiguousarray(
            tok.reshape(TPC, 3, 128).transpose(2, 1, 0)).astype(BF16)
        m = dict(common)
        m["xt"] = xt
        in_maps.append(m)
    return has_mask, in_maps


def kernel(**inputs):
    has_mask, in_maps = _prep_inputs(**inputs)
    nc = _get_nc(has_mask)
    res = run_bass_kernel_spmd(nc, in_maps, list(range(NCORES)))
    outs = []
    for core in range(NCORES):
        ot = res.results[core]["outT"]            # [128, 3, TPC] f32
        out = np.ascontiguousarray(ot.transpose(2, 1, 0)).reshape(TPC, D)
        outs.append(out)
    full = np.concatenate(outs, axis=0).reshape(B_, N, D)
    return full.astype(np.float32)


def run_traced(**inputs):
    """Like kernel() but with NTFF profiling; returns (out, BassKernelResults)."""
    sys.path.insert(0, "/root/problem")
    import profhook
    profhook.install()
    has_mask, in_maps = _prep_inputs(**inputs)
    nc = _get_nc(has_mask)
    res = run_bass_kernel_spmd(nc, in_maps, list(range(NCORES)), trace=True)
    outs = []
    for core in range(NCORES):
        ot = res.results[core]["outT"]
        out = np.ascontiguousarray(ot.transpose(2, 1, 0)).reshape(TPC, D)
        outs.append(out)
    full = np.concatenate(outs, axis=0).reshape(B_, N, D)
    return full.astype(np.float32), res



# revision 3
# speedup vs baseline: 1.1810x; 1.1810x over previous
"""Bass/Trainium2 kernel for nn_EnhancedPEFTWindowAttention.

Data-parallel over B_ (2048 windows*batch) across 8 NeuronCores:
256 windows = 12544 tokens per core. Weights / bias tables replicated.

Layout strategy (per core):
  - x is pre-transposed on host to channel-rows [128, 3, T] bf16 (main
    matmul) and fp8 DoubleRow-packed [128, 4, T] (gate/lora matmuls).
  - qkv LoRA-linear in channel-rows -> qkvT [128, 9, T] bf16, globally
    scaled x16 (absorbed by the exp activation scale and the host-side
    output unscale). Gate sigmoid is computed as (tanh(z/2)+1)/2 with
    the 1/2 folded into the lora-up weights, so the scalar engine only
    ever needs the exp/tanh activation table (no table reloads).
  - k and v output tiles carry no bias: the k bias is softmax-invariant
    (q.bk is constant over keys) and the v bias is folded into the proj
    layer's main/gate/lora biases analytically on the host.
  - Attention per window pair: scores S^T via 24 K=32 matmuls, softmax
    as exp(S*scale) * exp(bias+mask) with row sums from a ones-column
    appended to V, AV -> O token-rows, PE transposes -> O^T channel-rows
    (bf16 for the proj main matmul, fp8 for the proj gate/lora).
  - proj LoRA-linear in channel-rows -> outT [128, 3, T] f32 (x16),
    written transposed; host un-transposes and unscales.
"""

import sys

sys.path.insert(0, "/opt/trn_rl_repo")

import numpy as np
import ml_dtypes

import concourse.bacc as bacc
import concourse.tile as tile
from concourse import mybir
from concourse.bass_utils import run_bass_kernel_spmd

BF16 = ml_dtypes.bfloat16
NPFP8 = ml_dtypes.float8_e4m3

WS = 7
N = 49
H = 12
D = 384
HD = 32
NW = 64
B_ = 2048
R = 16
SCALING = 32.0 / 16.0
SCALE = HD ** -0.5

NCORES = 8
WPC = B_ // NCORES            # windows per core = 256
TPC = WPC * N                 # tokens per core = 12544
WCHUNK = 8                    # windows per chunk
TCHUNK = WCHUNK * N           # 392 tokens per chunk
NCHUNK = WPC // WCHUNK        # 32 chunks
NPAIR = WCHUNK // 2           # 4 pairs per chunk

F32 = mybir.dt.float32
BF = mybir.dt.bfloat16
FP8 = mybir.dt.float8e4
DR = mybir.MatmulPerfMode.DoubleRow

QS = 16.0                     # global scale on qkvT / oT / out
LIFT = 16.0                   # fp8 weight-quantization lift

_COMPILED = {}


def _build(has_mask: bool):
    nc = bacc.Bacc("TRN2", target_bir_lowering=False, debug=False,
                   num_devices=NCORES)

    def din(name, shape, dt):
        return nc.dram_tensor(name, shape, dt, kind="ExternalInput").ap()

    xt_d = din("xt", [128, 3, TPC], BF)
    xq_d = din("xq", [128, 4, TPC], FP8)
    wqkvT_d = din("wqkvT", [128, 3, 3 * D], BF)
    gateQ_d = din("gateQ", [128, 4, 3 * D], FP8)
    downQ_d = din("downQ", [128, 4, R], FP8)
    upQ_d = din("upQ", [R, 2, 3 * D], FP8)
    pwT_d = din("pwT", [128, 3, D], BF)
    pgateQ_d = din("pgateQ", [128, 4, D], FP8)
    pdownQ_d = din("pdownQ", [128, 4, R], FP8)
    pupQ_d = din("pupQ", [R, 2, D], FP8)
    biasQ_d = din("biasQ", [128, 12], F32)
    cgQ_d = din("cgQ", [128, 3], F32)
    cdQ_d = din("cdQ", [R, 1], F32)
    n_expb = 32 if has_mask else 1
    expb_d = din("expb", [n_expb, 113, 588], F32)
    ident_d = din("ident", [128, 128], BF)
    outT_d = nc.dram_tensor("outT", [128, 3, TPC], F32,
                            kind="ExternalOutput").ap()

    TANH = mybir.ActivationFunctionType.Tanh
    EXPF = mybir.ActivationFunctionType.Exp
    COPYF = mybir.ActivationFunctionType.Copy
    IDENT = mybir.ActivationFunctionType.Identity
    ADD = mybir.AluOpType.add
    MULT = mybir.AluOpType.mult

    with tile.TileContext(nc) as tc:
        consts = tc.alloc_tile_pool(name="consts", bufs=1)
        xt_p = tc.alloc_tile_pool(name="xt", bufs=3)
        qkvT_p = tc.alloc_tile_pool(name="qkvT", bufs=2)
        sb_p = tc.alloc_tile_pool(name="sb", bufs=3)
        attn_p = tc.alloc_tile_pool(name="attn", bufs=2)
        oT_p = tc.alloc_tile_pool(name="oT", bufs=2)
        out_p = tc.alloc_tile_pool(name="out", bufs=2)
        ps_lin = tc.alloc_tile_pool(name="ps_lin", bufs=3, space="PSUM")
        ps_s_p = tc.alloc_tile_pool(name="ps_s", bufs=1, space="PSUM")
        ps_o_p = tc.alloc_tile_pool(name="ps_o", bufs=1, space="PSUM")
        ps_vt_p = tc.alloc_tile_pool(name="ps_vt", bufs=1, space="PSUM")

        # ---- resident constants ----
        wqkvT = consts.tile([128, 3, 3 * D], BF)
        nc.sync.dma_start(out=wqkvT, in_=wqkvT_d[:])
        gateQ = consts.tile([128, 4, 3 * D], FP8)
        nc.sync.dma_start(out=gateQ, in_=gateQ_d[:])
        downQ = consts.tile([128, 4, R], FP8)
        nc.sync.dma_start(out=downQ, in_=downQ_d[:])
        upQ = consts.tile([R, 2, 3 * D], FP8)
        nc.sync.dma_start(out=upQ, in_=upQ_d[:])
        pwT = consts.tile([128, 3, D], BF)
        nc.sync.dma_start(out=pwT, in_=pwT_d[:])
        pgateQ = consts.tile([128, 4, D], FP8)
        nc.sync.dma_start(out=pgateQ, in_=pgateQ_d[:])
        pdownQ = consts.tile([128, 4, R], FP8)
        nc.sync.dma_start(out=pdownQ, in_=pdownQ_d[:])
        pupQ = consts.tile([R, 2, D], FP8)
        nc.sync.dma_start(out=pupQ, in_=pupQ_d[:])
        biasQ = consts.tile([128, 12], F32)
        nc.sync.dma_start(out=biasQ, in_=biasQ_d[:])
        cgQ = consts.tile([128, 3], F32)
        nc.sync.dma_start(out=cgQ, in_=cgQ_d[:])
        cdQ = consts.tile([R, 1], F32)
        nc.sync.dma_start(out=cdQ, in_=cdQ_d[:])
        ident = consts.tile([128, 128], BF)
        nc.sync.dma_start(out=ident, in_=ident_d[:])
        expb_c = None
        if not has_mask:
            expb_c = consts.tile([113, 2, 294], F32)
            nc.sync.dma_start(out=expb_c, in_=expb_d[0])

        # persistent attention tiles
        ps_s = ps_s_p.tile([113, 2, 512], F32, tag="s")
        nc.vector.memset(ps_s[32:64, :, 0:294], 0.0)
        v_exts = []
        for i in range(2):
            ve = consts.tile([113, H, HD + 1], BF, name=f"v_ext{i}")
            nc.vector.memset(ve[:, :, HD:HD + 1], 1.0)
            v_exts.append(ve)
        o_sbs = []
        for i in range(2):
            ob = consts.tile([113, H, HD], BF, name=f"o_sb{i}")
            nc.vector.memset(ob[32:64, :, :], 0.0)
            o_sbs.append(ob)

        qkvT_tiles = {}
        oT_tiles = {}
        oTq_tiles = {}

        def emit_A(c):
            """qkv LoRA-linear for chunk c; yields after xd and each m-tile."""
            t0 = c * TCHUNK
            xt = xt_p.tile([128, 3, TCHUNK], BF, name="xt")
            nc.sync.dma_start(out=xt, in_=xt_d[:, :, t0:t0 + TCHUNK])
            xq = xt_p.tile([128, 4, TCHUNK], FP8, name="xq")
            nc.sync.dma_start(out=xq, in_=xq_d[:, :, t0:t0 + TCHUNK])
            qkvT = qkvT_p.tile([128, 9, TCHUNK], BF, name="qkvT")
            qkvT_tiles[c] = qkvT

            # xd = down @ x (fp8 DoubleRow, psum = 16*xd)
            ps_xd = ps_lin.tile([128, 512], F32, tag="lin", name="ps_xd")[0:R, 0:TCHUNK]
            nc.tensor.matmul(ps_xd, downQ[:, 0:2, :], xq[:, 0:2, :],
                             start=True, stop=False, perf_mode=DR)
            nc.tensor.matmul(ps_xd, downQ[:, 2:4, :], xq[:, 2:4, :],
                             start=False, stop=True, perf_mode=DR)
            xd = sb_p.tile([R, 2, TCHUNK], FP8, tag="xd", name="xd")
            nc.gpsimd.memset(xd[:, 1, :], 0.0)
            nc.scalar.activation(xd[:, 0, :], ps_xd[:], IDENT,
                                 scale=float(1.0 / LIFT))
            yield

            for mi in range(9):
                ps_g = ps_lin.tile([128, 512], F32, tag="lin", name="ps_g")[:, 0:TCHUNK]
                nc.tensor.matmul(ps_g, gateQ[:, 0:2, 128 * mi:128 * mi + 128],
                                 xq[:, 0:2, :], start=True, stop=False,
                                 perf_mode=DR)
                nc.tensor.matmul(ps_g, gateQ[:, 2:4, 128 * mi:128 * mi + 128],
                                 xq[:, 2:4, :], start=False, stop=True,
                                 perf_mode=DR)
                t_sb = sb_p.tile([128, TCHUNK], BF, tag="g", name="t_sb")
                # ps_g = LIFT*z  ->  tanh(z/2)
                nc.scalar.activation(t_sb[:], ps_g[:], TANH,
                                     scale=float(0.5 / LIFT))

                ps_l = ps_lin.tile([128, 512], F32, tag="lin", name="ps_l")[:, 0:TCHUNK]
                nc.tensor.matmul(ps_l, upQ[:, :, 128 * mi:128 * mi + 128],
                                 xd[:], start=True, stop=True, perf_mode=DR)

                ps_m = ps_lin.tile([128, 512], F32, tag="lin", name="ps_m")[:, 0:TCHUNK]
                for ki in range(3):
                    nc.tensor.matmul(ps_m, wqkvT[:, ki, 128 * mi:128 * mi + 128],
                                     xt[:, ki, :],
                                     start=(ki == 0), stop=(ki == 2))
                gl = sb_p.tile([128, TCHUNK], F32, tag="gl", name="gl")
                # gl = (t + 1) * ps_l    (the 1/2 lives in the up weights)
                nc.vector.scalar_tensor_tensor(gl[:], t_sb[:], 1.0, ps_l[:],
                                               ADD, MULT)
                # out = (ps_m + bias) + gl
                nc.vector.scalar_tensor_tensor(qkvT[:, mi, :], ps_m[:],
                                               biasQ[:, mi:mi + 1], gl[:],
                                               ADD, ADD)
                yield

        def emit_B_front(c, p):
            """attention for pair p of chunk c (through AV + normalize)."""
            qkvT = qkvT_tiles[c]
            if p == 0:
                oT_tiles[c] = oT_p.tile([128, 3, TCHUNK], BF, name="oT")
                oTq = oT_p.tile([128, 4, TCHUNK], FP8, name="oTq")
                nc.gpsimd.memset(oTq[:, 3, :], 0.0)
                oTq_tiles[c] = oTq
            pc0 = p * 2 * N
            if has_mask:
                expb = attn_p.tile([113, 2, 294], F32, tag="expb", name="expb")
                pm = (c * NPAIR + p) % 32
                nc.sync.dma_start(out=expb, in_=expb_d[pm])
            else:
                expb = expb_c

            # ---- hardware constraint driving the structure below:
            # concurrently-in-flight matmuls on different PE row groups
            # must write different PSUM banks. QK heads live at 4 row
            # bases (32*(h%4)); we run row groups {0,1} (banks 0,1),
            # then the full-K V-transposes (which conflict with every
            # row group, acting as a PE barrier), then groups {2,3}
            # reusing banks 0,1. AV w0/w1 use row groups {0,1}/{2,3}
            # and are split across banks by w.
            scol = lambda h: 49 * (3 * ((h % 4) // 2) + h // 4)

            # QK batch 1: heads with h%4 in {0,1}
            qk_b1 = []
            for w in range(2):
                wc0 = pc0 + w * N
                for h in range(12):
                    if h % 4 >= 2:
                        continue
                    mm = nc.tensor.matmul(
                        ps_s[64 * w:64 * w + N, h % 2,
                             scol(h):scol(h) + 49],
                        qkvT[32 * (h % 4):32 * (h % 4) + 32, 3 + h // 4,
                             wc0:wc0 + N],
                        qkvT[32 * (h % 4):32 * (h % 4) + 32, h // 4,
                             wc0:wc0 + N],
                        start=True, stop=True,
                        tile_position=(32 * (h % 4), 64 * w))
                    qk_b1.append(mm.ins)

            # V transposes (full-K: PE barrier between QK batches)
            v_ext = v_exts[p % 2]
            vt_insts = []
            ps_vt = ps_vt_p.tile([113, 3, 128], BF, tag="vt", name="ps_vt")
            for w in range(2):
                wc0 = pc0 + w * N
                for ki in range(3):
                    t = nc.tensor.transpose(
                        ps_vt[64 * w:64 * w + N, ki, :],
                        qkvT[:, 6 + ki, wc0:wc0 + N],
                        ident[:],
                        tile_position=(0, 64 * w))
                    for prev in qk_b1:
                        tile.add_dep_helper(t.ins, prev,
                                            reason="qk-b1 before vt")
                    vt_insts.append(t.ins)
            for w in range(2):
                nc.scalar.copy(
                    v_ext[64 * w:64 * w + N, :, 0:HD],
                    ps_vt[64 * w:64 * w + N, :, :].rearrange(
                        "p a (b c) -> p (a b) c", c=HD))

            # QK batch 2: heads with h%4 in {2,3}, after the barrier
            for w in range(2):
                wc0 = pc0 + w * N
                for h in range(12):
                    if h % 4 < 2:
                        continue
                    mm = nc.tensor.matmul(
                        ps_s[64 * w:64 * w + N, h % 2,
                             scol(h):scol(h) + 49],
                        qkvT[32 * (h % 4):32 * (h % 4) + 32, 3 + h // 4,
                             wc0:wc0 + N],
                        qkvT[32 * (h % 4):32 * (h % 4) + 32, h // 4,
                             wc0:wc0 + N],
                        start=True, stop=True,
                        tile_position=(32 * (h % 4), 64 * w))
                    for vt in vt_insts:
                        tile.add_dep_helper(mm.ins, vt,
                                            reason="vt before qk-b2")

            # exp directly from PSUM; ps_s = QS^2/SCALE^-1... = QS^2 * S/scale
            e_sb = attn_p.tile([113, 2, 294], F32, tag="e", name="e_sb")
            nc.scalar.activation(e_sb[:], ps_s[:, :, 0:294], EXPF,
                                 scale=float(SCALE / (QS * QS)))
            ep = attn_p.tile([113, 2, 294], BF, tag="ep", name="ep")
            nc.vector.tensor_tensor(ep[:], e_sb[:], expb[:],
                                    mybir.AluOpType.mult)

            # AV (+ rowsum in col 32 via ones column of v_ext);
            # psum bank = w so w0/w1 row-group sets never share a bank
            ps_o_t = ps_o_p.tile([113, 2, 512], F32, tag="o", name="ps_o")
            ps_o = [
                ps_o_t[64 * w:64 * w + N, w, 0:H * (HD + 1)].rearrange(
                    "p (h c) -> p h c", c=HD + 1)
                for w in range(2)
            ]
            for w in range(2):
                for h in range(12):
                    nc.tensor.matmul(
                        ps_o[w][:, h, :],
                        ep[64 * w:64 * w + N, h % 2,
                           scol(h):scol(h) + 49],
                        v_ext[64 * w:64 * w + N, h, :],
                        start=True, stop=True,
                        tile_position=(64 * w, 64 * w))

            r = attn_p.tile([113, H, 1], F32, tag="r", name="r")
            for w in range(2):
                nc.vector.reciprocal(r[64 * w:64 * w + N, :, :],
                                     ps_o[w][:, :, HD:HD + 1])
            o_sb = o_sbs[p % 2]
            for w in range(2):
                nc.vector.tensor_tensor(
                    o_sb[64 * w:64 * w + N, :, :],
                    ps_o[w][:, :, 0:HD],
                    r[64 * w:64 * w + N, :, :].to_broadcast([N, H, HD]),
                    mybir.AluOpType.mult)

        def emit_B_back(c, p):
            oT = oT_tiles[c]
            oTq = oTq_tiles[c]
            pc0 = p * 2 * N
            o_sb = o_sbs[p % 2]
            # O^T: transpose [113, 128] -> [128, 113], copy valid halves
            ps_ot = ps_vt_p.tile([128, 3, 128], BF, tag="vt", name="ps_ot")
            for ki in range(3):
                nc.tensor.transpose(ps_ot[:, ki, 0:113],
                                    o_sb[:, 4 * ki:4 * ki + 4, :],
                                    ident[0:113, 0:113])
            for w in range(2):
                nc.scalar.copy(
                    oT[:, :, pc0 + w * N:pc0 + (w + 1) * N],
                    ps_ot[:, :, 64 * w:64 * w + N])
                nc.scalar.copy(
                    oTq[:, 0:3, pc0 + w * N:pc0 + (w + 1) * N],
                    ps_ot[:, :, 64 * w:64 * w + N])

        def emit_C(c):
            """proj LoRA-linear for chunk c; yields after pxd and each m-tile."""
            t0 = c * TCHUNK
            oT = oT_tiles.pop(c)
            oTq = oTq_tiles.pop(c)
            qkvT_tiles.pop(c, None)
            ps_pxd = ps_lin.tile([128, 512], F32, tag="lin", name="ps_pxd")[0:R, 0:TCHUNK]
            nc.tensor.matmul(ps_pxd, pdownQ[:, 0:2, :], oTq[:, 0:2, :],
                             start=True, stop=False, perf_mode=DR)
            nc.tensor.matmul(ps_pxd, pdownQ[:, 2:4, :], oTq[:, 2:4, :],
                             start=False, stop=True, perf_mode=DR)
            pxd = sb_p.tile([R, 2, TCHUNK], FP8, tag="xd", name="pxd")
            nc.gpsimd.memset(pxd[:, 1, :], 0.0)
            # ps_pxd = LIFT*QS * xd2; Identity allows the per-partition bias
            nc.scalar.activation(pxd[:, 0, :], ps_pxd[:], IDENT,
                                 bias=cdQ[:], scale=float(1.0 / (LIFT * QS)))
            yield

            out_sb = out_p.tile([128, 3, TCHUNK], F32, name="out_sb")
            for mi in range(3):
                ps_g2 = ps_lin.tile([128, 512], F32, tag="lin", name="ps_g2")[:, 0:TCHUNK]
                nc.tensor.matmul(ps_g2, pgateQ[:, 0:2, 128 * mi:128 * mi + 128],
                                 oTq[:, 0:2, :], start=True, stop=False,
                                 perf_mode=DR)
                nc.tensor.matmul(ps_g2, pgateQ[:, 2:4, 128 * mi:128 * mi + 128],
                                 oTq[:, 2:4, :], start=False, stop=True,
                                 perf_mode=DR)
                t2 = sb_p.tile([128, TCHUNK], BF, tag="g", name="t2")
                # ps_g2 = LIFT*QS*z2 -> tanh((z2+cg)/2)
                nc.scalar.activation(t2[:], ps_g2[:], TANH,
                                     bias=cgQ[:, mi:mi + 1],
                                     scale=float(0.5 / (LIFT * QS)))

                ps_l2 = ps_lin.tile([128, 512], F32, tag="lin", name="ps_l2")[:, 0:TCHUNK]
                nc.tensor.matmul(ps_l2, pupQ[:, :, 128 * mi:128 * mi + 128],
                                 pxd[:], start=True, stop=True, perf_mode=DR)

                ps_m2 = ps_lin.tile([128, 512], F32, tag="lin", name="ps_m2")[:, 0:TCHUNK]
                for ki in range(3):
                    nc.tensor.matmul(ps_m2, pwT[:, ki, 128 * mi:128 * mi + 128],
                                     oT[:, ki, :],
                                     start=(ki == 0), stop=(ki == 2))
                gl2 = sb_p.tile([128, TCHUNK], F32, tag="gl", name="gl2")
                nc.vector.scalar_tensor_tensor(gl2[:], t2[:], 1.0, ps_l2[:],
                                               ADD, MULT)
                nc.vector.scalar_tensor_tensor(out_sb[:, mi, :], ps_m2[:],
                                               biasQ[:, 9 + mi:10 + mi],
                                               gl2[:], ADD, ADD)
                yield

            nc.sync.dma_start(out=outT_d[:, :, t0:t0 + TCHUNK], in_=out_sb)


        # clean schedule: for each c, emit A(c) m-tiles interleaved with
        # B pairs of c-1, then C(c-1) m-tiles interleaved into A(c) tail.
        prev_b = None  # chunk index whose B/C still need emission
        for c in range(NCHUNK):
            a_gen = emit_A(c)
            plan = []
            if prev_b is not None:
                for p in range(NPAIR):
                    plan.append(("Bf", (prev_b, p)))
                    plan.append(("Bb", (prev_b, p)))
                plan.append(("C", prev_b))
            # a_gen yields 10 times (xd + 9 m-tiles)
            c_sub = None
            for step in range(10):
                try:
                    next(a_gen)
                except StopIteration:
                    break
                if step == 0:
                    continue
                if plan:
                    kind, arg = plan[0]
                    if kind == "Bf":
                        emit_B_front(*arg)
                        plan.pop(0)
                    elif kind == "Bb":
                        emit_B_back(*arg)
                        plan.pop(0)
                    else:
                        if c_sub is None:
                            c_sub = emit_C(arg)
                        try:
                            next(c_sub)
                        except StopIteration:
                            c_sub = None
                            plan.pop(0)
            # finish any leftover B/C of prev chunk
            while plan:
                kind, arg = plan.pop(0)
                if kind == "Bf":
                    emit_B_front(*arg)
                elif kind == "Bb":
                    emit_B_back(*arg)
                else:
                    if c_sub is None:
                        c_sub = emit_C(arg)
                    for _ in c_sub:
                        pass
                    c_sub = None
            if c_sub is not None:
                for _ in c_sub:
                    pass
                c_sub = None
            prev_b = c
        # trailing B + C of the last chunk
        for p in range(NPAIR):
            emit_B_front(prev_b, p)
            emit_B_back(prev_b, p)
        for _ in emit_C(prev_b):
            pass

        for pool in reversed((consts, xt_p, qkvT_p, sb_p, attn_p, oT_p, out_p,
                              ps_lin, ps_s_p, ps_o_p, ps_vt_p)):
            pool.release()

    nc.compile()
    return nc


def _get_nc(has_mask: bool):
    if has_mask not in _COMPILED:
        _COMPILED[has_mask] = _build(has_mask)
    return _COMPILED[has_mask]


def _arr_lhsT(w_t, kparts):
    """[K, M] -> [128, K//128, M] partition-tiled lhsT layout."""
    K, M = w_t.shape
    return np.ascontiguousarray(
        w_t.reshape(kparts, 128, M).transpose(1, 0, 2))


def _pack_dr(w_t):
    """[K<=384, M] f32 -> [128, 4, M] DoubleRow-planes (plane 3 zeros)."""
    K, M = w_t.shape
    out = np.zeros((128, 4, M), np.float32)
    for plane in range(4):
        lo = plane * 128
        hi = min(lo + 128, K)
        if lo < K:
            out[0:hi - lo, plane] = w_t[lo:hi]
    return out


def _q8(a):
    return np.asarray(a, np.float32).astype(NPFP8)


def _prep_inputs(x, mask, qkv_w, qkv_b, qkv_down, qkv_up, qkv_gate, qkv_res,
                 proj_w, proj_b, proj_down, proj_up, proj_gate, proj_res,
                 bias_table, rel_index):
    x = np.asarray(x, np.float32)
    mask = np.asarray(mask, np.float32)
    has_mask = bool(np.any(mask))

    qkv_b = np.asarray(qkv_b, np.float32)
    proj_b = np.asarray(proj_b, np.float32)
    w_eff = (np.asarray(qkv_w, np.float32)
             + np.asarray(qkv_res, np.float32))        # [1152, 384]
    pw_eff = (np.asarray(proj_w, np.float32)
              + np.asarray(proj_res, np.float32))      # [384, 384]
    pg = np.asarray(proj_gate, np.float32)
    pd = np.asarray(proj_down, np.float32)

    bv = qkv_b[2 * D:3 * D]                            # v bias, folded below
    bp_eff = proj_b + bv @ pw_eff.T                    # main-path fold
    cg = bv @ pg.T                                     # gate-path fold
    cd = bv @ pd.T                                     # lora-down fold

    biasQ = np.zeros((128, 12), np.float32)
    for mi in range(3):                                # q bias only (k/v: none)
        biasQ[:, mi] = QS * qkv_b[128 * mi:128 * mi + 128]
    for mi in range(3):
        biasQ[:, 9 + mi] = QS * bp_eff[128 * mi:128 * mi + 128]

    common = {
        "wqkvT": _arr_lhsT((QS * w_eff).T, 3).astype(BF16),
        "gateQ": _q8(_pack_dr(LIFT * np.asarray(qkv_gate, np.float32).T)),
        "downQ": _q8(_pack_dr(LIFT * np.asarray(qkv_down, np.float32).T)),
        "pwT": _arr_lhsT(pw_eff.T, 3).astype(BF16),
        "pgateQ": _q8(_pack_dr(LIFT * pg.T)),
        "pdownQ": _q8(_pack_dr(LIFT * pd.T)),
        "biasQ": biasQ,
        "cgQ": np.ascontiguousarray(
            0.5 * cg.reshape(3, 128).T.astype(np.float32)),
        "cdQ": cd.reshape(R, 1).astype(np.float32),
        "ident": np.eye(128, dtype=BF16),
    }
    # lora-up weights carry the tanh 1/2: 0.5*SCALING = 1.0
    upq = np.zeros((R, 2, 3 * D), np.float32)
    upq[:, 0, :] = LIFT * (0.5 * SCALING) * np.asarray(qkv_up, np.float32).T
    common["upQ"] = _q8(upq)
    pupq = np.zeros((R, 2, D), np.float32)
    pupq[:, 0, :] = LIFT * (0.5 * SCALING) * np.asarray(proj_up, np.float32).T
    common["pupQ"] = _q8(pupq)

    # exp(relative-position bias + mask) in S^T layout [m, bank*294+scol(h)+n]
    bt = np.asarray(bias_table, np.float32)
    ri = np.asarray(rel_index).astype(np.int64)
    b_nmh = bt[ri]                                # [n, m, H]
    hperm = np.argsort(
        [(h % 2) * 6 + 3 * ((h % 4) // 2) + h // 4 for h in range(H)])

    def _eT(lg):                                  # lg [n, m, H] -> [m, 588]
        e = np.exp(lg).transpose(1, 2, 0)         # [m, H, n]
        return e[:, hperm, :].reshape(N, 588)

    if has_mask:
        expb = np.zeros((32, 113, 588), np.float32)
        for pm in range(32):
            for w01 in range(2):
                lg = b_nmh + mask[2 * pm + w01][:, :, None]   # [n, m, H]
                expb[pm, 64 * w01:64 * w01 + N] = _eT(lg)
    else:
        expb = np.zeros((1, 113, 588), np.float32)
        eT = _eT(b_nmh)
        expb[0, 0:N] = eT
        expb[0, 64:64 + N] = eT
    common["expb"] = expb

    in_maps = []
    for core in range(NCORES):
        tok = np.ascontiguousarray(
            x[core * WPC:(core + 1) * WPC].reshape(TPC, D))
        xt = np.ascontiguousarray(
            tok.reshape(TPC, 3, 128).transpose(2, 1, 0)).astype(BF16)
        xq = np.zeros((128, 4, TPC), NPFP8)
        xq[:, 0:3, :] = xt.astype(np.float32).astype(NPFP8)
        m = dict(common)
        m["xt"] = xt
        m["xq"] = xq
        in_maps.append(m)
    return has_mask, in_maps


def _gather(res):
    outs = []
    for core in range(NCORES):
        ot = res.results[core]["outT"]            # [128, 3, TPC] f32 (x QS)
        out = np.ascontiguousarray(ot.transpose(2, 1, 0)).reshape(TPC, D)
        outs.append(out)
    full = np.concatenate(outs, axis=0).reshape(B_, N, D)
    return (full * (1.0 / QS)).astype(np.float32)


def kernel(**inputs):
    has_mask, in_maps = _prep_inputs(**inputs)
    nc = _get_nc(has_mask)
    res = run_bass_kernel_spmd(nc, in_maps, list(range(NCORES)))
    return _gather(res)


def run_traced(**inputs):
    """Like kernel() but with NTFF profiling; returns (out, BassKernelResults)."""
    sys.path.insert(0, "/root/problem")
    import profhook
    profhook.install()
    has_mask, in_maps = _prep_inputs(**inputs)
    nc = _get_nc(has_mask)
    res = run_bass_kernel_spmd(nc, in_maps, list(range(NCORES)), trace=True)
    return _gather(res), res


# revision 28
# speedup vs baseline: 1.5382x; 1.3024x over previous
"""Bass/Trainium2 kernel for nn_EnhancedPEFTWindowAttention.

Data-parallel over B_ (2048 windows*batch) across 8 NeuronCores:
256 windows = 12544 tokens per core. Weights / bias tables replicated.

Layout strategy (per core):
  - x is pre-transposed on host to channel-rows [128, 3, T] bf16 (main
    matmul) and fp8 DoubleRow-packed [128, 4, T] (gate/lora matmuls).
  - qkv LoRA-linear in channel-rows -> qkvT [128, 9, T] bf16, globally
    scaled x16 (absorbed by the exp activation scale and the host-side
    output unscale). Gate sigmoid is computed as (tanh(z/2)+1)/2 with
    the 1/2 folded into the lora-up weights, so the scalar engine only
    ever needs the exp/tanh activation table (no table reloads).
  - k and v output tiles carry no bias: the k bias is softmax-invariant
    (q.bk is constant over keys) and the v bias is folded into the proj
    layer's main/gate/lora biases analytically on the host.
  - Attention per window pair: scores S^T via 24 K=32 matmuls, softmax
    as exp(S*scale) * exp(bias+mask) with row sums from a ones-column
    appended to V, AV -> O token-rows, PE transposes -> O^T channel-rows
    (bf16 for the proj main matmul, fp8 for the proj gate/lora).
  - proj LoRA-linear in channel-rows -> outT [128, 3, T] f32 (x16),
    written transposed; host un-transposes and unscales.
"""

import sys

sys.path.insert(0, "/opt/trn_rl_repo")

import numpy as np
import ml_dtypes

import concourse.bacc as bacc
import concourse.tile as tile
from concourse import mybir
from concourse.bass_utils import run_bass_kernel_spmd

BF16 = ml_dtypes.bfloat16
NPFP8 = ml_dtypes.float8_e4m3

WS = 7
N = 49
H = 12
D = 384
HD = 32
NW = 64
B_ = 2048
R = 16
SCALING = 32.0 / 16.0
SCALE = HD ** -0.5

NCORES = 8
WPC = B_ // NCORES            # windows per core = 256
TPC = WPC * N                 # tokens per core = 12544
WCHUNK = 8                    # windows per chunk
TCHUNK = WCHUNK * N           # 392 tokens per chunk
NCHUNK = WPC // WCHUNK        # 32 chunks
NPAIR = WCHUNK // 2           # 4 pairs per chunk

F32 = mybir.dt.float32
BF = mybir.dt.bfloat16
FP8 = mybir.dt.float8e4
DR = mybir.MatmulPerfMode.DoubleRow

QS = 16.0                     # global scale on qkvT / oT / out
LIFT = 16.0                   # fp8 weight-quantization lift

_COMPILED = {}


def _build(has_mask: bool):
    nc = bacc.Bacc("TRN2", target_bir_lowering=False, debug=False,
                   num_devices=NCORES)

    def din(name, shape, dt):
        return nc.dram_tensor(name, shape, dt, kind="ExternalInput").ap()

    xt_d = din("xt", [128, 3, TPC], BF)
    xq_d = din("xq", [128, 4, TPC], FP8)
    wqkvT_d = din("wqkvT", [128, 3, 3 * D], BF)
    gateQ_d = din("gateQ", [128, 4, 3 * D], FP8)
    downQ_d = din("downQ", [128, 4, R], FP8)
    upQ_d = din("upQ", [R, 2, 3 * D], FP8)
    pwT_d = din("pwT", [128, 3, D], BF)
    pgateQ_d = din("pgateQ", [128, 4, D], FP8)
    pdownQ_d = din("pdownQ", [128, 4, R], FP8)
    pupQ_d = din("pupQ", [R, 2, D], FP8)
    biasQ_d = din("biasQ", [128, 12], F32)
    cgQ_d = din("cgQ", [128, 3], F32)
    cdQ_d = din("cdQ", [R, 1], F32)
    n_expb = 32 if has_mask else 1
    expb_d = din("expb", [n_expb, 113, 588], F32)
    ident_d = din("ident", [128, 128], BF)
    outT_d = nc.dram_tensor("outT", [128, 3, TPC], F32,
                            kind="ExternalOutput").ap()

    TANH = mybir.ActivationFunctionType.Tanh
    EXPF = mybir.ActivationFunctionType.Exp
    COPYF = mybir.ActivationFunctionType.Copy
    IDENT = mybir.ActivationFunctionType.Identity
    ADD = mybir.AluOpType.add
    MULT = mybir.AluOpType.mult

    with tile.TileContext(nc) as tc:
        consts = tc.alloc_tile_pool(name="consts", bufs=1)
        xt_p = tc.alloc_tile_pool(name="xt", bufs=3)
        qkvT_p = tc.alloc_tile_pool(name="qkvT", bufs=2)
        sb_p = tc.alloc_tile_pool(name="sb", bufs=3)
        attn_p = tc.alloc_tile_pool(name="attn", bufs=2)
        oT_p = tc.alloc_tile_pool(name="oT", bufs=2)
        out_p = tc.alloc_tile_pool(name="out", bufs=2)
        ps_lin = tc.alloc_tile_pool(name="ps_lin", bufs=3, space="PSUM")
        ps_s_p = tc.alloc_tile_pool(name="ps_s", bufs=1, space="PSUM")
        ps_o_p = tc.alloc_tile_pool(name="ps_o", bufs=1, space="PSUM")
        ps_vt_p = tc.alloc_tile_pool(name="ps_vt", bufs=1, space="PSUM")

        # ---- resident constants ----
        # DMA order matters at startup: the first chunk's down/gate/main
        # matmuls need downQ/gateQ/upQ/wqkvT; proj-side consts come last.
        downQ = consts.tile([128, 4, R], FP8)
        nc.sync.dma_start(out=downQ, in_=downQ_d[:])
        upQ = consts.tile([R, 2, 3 * D], FP8)
        nc.sync.dma_start(out=upQ, in_=upQ_d[:])
        biasQ = consts.tile([128, 12], F32)
        nc.sync.dma_start(out=biasQ, in_=biasQ_d[:])
        gateQ = consts.tile([128, 4, 3 * D], FP8)
        nc.sync.dma_start(out=gateQ, in_=gateQ_d[:])
        wqkvT = consts.tile([128, 3, 3 * D], BF)
        nc.sync.dma_start(out=wqkvT, in_=wqkvT_d[:])
        ident = consts.tile([128, 128], BF)
        nc.sync.dma_start(out=ident, in_=ident_d[:])
        pwT = consts.tile([128, 3, D], BF)
        nc.sync.dma_start(out=pwT, in_=pwT_d[:])
        pgateQ = consts.tile([128, 4, D], FP8)
        nc.sync.dma_start(out=pgateQ, in_=pgateQ_d[:])
        pdownQ = consts.tile([128, 4, R], FP8)
        nc.sync.dma_start(out=pdownQ, in_=pdownQ_d[:])
        pupQ = consts.tile([R, 2, D], FP8)
        nc.sync.dma_start(out=pupQ, in_=pupQ_d[:])
        cgQ = consts.tile([128, 3], F32)
        nc.sync.dma_start(out=cgQ, in_=cgQ_d[:])
        cdQ = consts.tile([R, 1], F32)
        nc.sync.dma_start(out=cdQ, in_=cdQ_d[:])
        expb_c = None
        if not has_mask:
            expb_c = consts.tile([113, 2, 294], F32)
            nc.sync.dma_start(out=expb_c, in_=expb_d[0])

        # persistent attention tiles
        ps_s = ps_s_p.tile([113, 2, 512], F32, tag="s")
        nc.vector.memset(ps_s[32:64, :, 0:294], 0.0)
        v_exts = []
        for i in range(2):
            ve = consts.tile([113, H, HD + 1], BF, name=f"v_ext{i}")
            nc.vector.memset(ve[:, :, HD:HD + 1], 1.0)
            v_exts.append(ve)
        o_sbs = []
        for i in range(2):
            ob = consts.tile([113, H, HD], BF, name=f"o_sb{i}")
            nc.vector.memset(ob[32:64, :, :], 0.0)
            o_sbs.append(ob)

        qkvT_tiles = {}
        oT_tiles = {}
        oTq_tiles = {}

        def emit_A(c):
            """qkv LoRA-linear for chunk c; yields after xd and each m-tile."""
            t0 = c * TCHUNK
            xt = xt_p.tile([128, 3, TCHUNK], BF, name="xt")
            nc.sync.dma_start(out=xt, in_=xt_d[:, :, t0:t0 + TCHUNK])
            xq = xt_p.tile([128, 4, TCHUNK], FP8, name="xq")
            nc.sync.dma_start(out=xq, in_=xq_d[:, :, t0:t0 + TCHUNK])
            qkvT = qkvT_p.tile([128, 6, TCHUNK], BF, name="qkvT")
            # v in window-padded layout (64 cols per window) so one PE
            # transpose covers a whole window pair with w1 tokens landing
            # at partition 64+ (pad cols transpose to ignored rows)
            vpad = qkvT_p.tile([128, 3, WCHUNK * 64], BF, name="vpad")
            qkvT_tiles[c] = (qkvT, vpad)

            # xd = down @ x (fp8 DoubleRow, psum = 16*xd)
            ps_xd = ps_lin.tile([128, 512], F32, tag="lin", name="ps_xd")[0:R, 0:TCHUNK]
            nc.tensor.matmul(ps_xd, downQ[:, 0:2, :], xq[:, 0:2, :],
                             start=True, stop=False, perf_mode=DR)
            nc.tensor.matmul(ps_xd, downQ[:, 2:4, :], xq[:, 2:4, :],
                             start=False, stop=True, perf_mode=DR)
            xd = sb_p.tile([R, 2, TCHUNK], FP8, tag="xd", name="xd")
            nc.gpsimd.memset(xd[:, 1, :], 0.0)
            nc.scalar.activation(xd[:, 0, :], ps_xd[:], IDENT,
                                 scale=float(1.0 / LIFT))
            yield

            for mi in range(9):
                ps_g = ps_lin.tile([128, 512], F32, tag="lin", name="ps_g")[:, 0:TCHUNK]
                nc.tensor.matmul(ps_g, gateQ[:, 0:2, 128 * mi:128 * mi + 128],
                                 xq[:, 0:2, :], start=True, stop=False,
                                 perf_mode=DR)
                nc.tensor.matmul(ps_g, gateQ[:, 2:4, 128 * mi:128 * mi + 128],
                                 xq[:, 2:4, :], start=False, stop=True,
                                 perf_mode=DR)
                t_sb = sb_p.tile([128, TCHUNK], BF, tag="g", name="t_sb")
                # ps_g = LIFT*z  ->  tanh(z/2)
                nc.scalar.activation(t_sb[:], ps_g[:], TANH,
                                     scale=float(0.5 / LIFT))

                ps_l = ps_lin.tile([128, 512], F32, tag="lin", name="ps_l")[:, 0:TCHUNK]
                nc.tensor.matmul(ps_l, upQ[:, :, 128 * mi:128 * mi + 128],
                                 xd[:], start=True, stop=True, perf_mode=DR)

                ps_m = ps_lin.tile([128, 512], F32, tag="lin", name="ps_m")[:, 0:TCHUNK]
                for ki in range(3):
                    nc.tensor.matmul(ps_m, wqkvT[:, ki, 128 * mi:128 * mi + 128],
                                     xt[:, ki, :],
                                     start=(ki == 0), stop=(ki == 2))
                gl = sb_p.tile([128, TCHUNK], F32, tag="gl", name="gl")
                # gl = (t + 1) * ps_l    (the 1/2 lives in the up weights)
                nc.vector.scalar_tensor_tensor(gl[:], t_sb[:], 1.0, ps_l[:],
                                               ADD, MULT)
                # out = (ps_m + bias) + gl
                if mi >= 6:
                    dst = vpad[:, mi - 6, :].rearrange(
                        "p (w n) -> p w n", n=64)[:, :, 0:N]
                    nc.vector.scalar_tensor_tensor(
                        dst, ps_m.rearrange("p (w n) -> p w n", n=N),
                        biasQ[:, mi:mi + 1],
                        gl[:].rearrange("p (w n) -> p w n", n=N),
                        ADD, ADD)
                else:
                    nc.vector.scalar_tensor_tensor(qkvT[:, mi, :], ps_m[:],
                                                   biasQ[:, mi:mi + 1], gl[:],
                                                   ADD, ADD)
                yield

        def emit_B_front(c, p):
            """attention for pair p of chunk c (through AV + normalize)."""
            qkvT, vpad = qkvT_tiles[c]
            if p == 0:
                oT_tiles[c] = oT_p.tile([128, 3, TCHUNK], BF, name="oT")
                oTq = oT_p.tile([128, 4, TCHUNK], FP8, name="oTq")
                nc.gpsimd.memset(oTq[:, 3, :], 0.0)
                oTq_tiles[c] = oTq
            pc0 = p * 2 * N
            if has_mask:
                expb = attn_p.tile([113, 2, 294], F32, tag="expb", name="expb")
                pm = (c * NPAIR + p) % 32
                nc.sync.dma_start(out=expb, in_=expb_d[pm])
            else:
                expb = expb_c

            # ---- hardware constraint driving the structure below:
            # concurrently-in-flight matmuls on different PE row groups
            # must write different PSUM banks. QK heads live at 4 row
            # bases (32*(h%4)); we run row groups {0,1} (banks 0,1),
            # then the full-K V-transposes (which conflict with every
            # row group, acting as a PE barrier), then groups {2,3}
            # reusing banks 0,1. AV w0/w1 use row groups {0,1}/{2,3}
            # and are split across banks by w.
            scol = lambda h: 49 * (3 * ((h % 4) // 2) + h // 4)

            # QK batch 1: heads with h%4 in {0,1}
            qk_b1 = []
            for w in range(2):
                wc0 = pc0 + w * N
                for h in range(12):
                    if h % 4 >= 2:
                        continue
                    mm = nc.tensor.matmul(
                        ps_s[64 * w:64 * w + N, h % 2,
                             scol(h):scol(h) + 49],
                        qkvT[32 * (h % 4):32 * (h % 4) + 32, 3 + h // 4,
                             wc0:wc0 + N],
                        qkvT[32 * (h % 4):32 * (h % 4) + 32, h // 4,
                             wc0:wc0 + N],
                        start=True, stop=True,
                        tile_position=(32 * (h % 4), 64 * w))
                    qk_b1.append(mm.ins)

            # V transposes (full-K: PE barrier between QK batches).
            # vpad's 64-token window stride puts w1 tokens at rows 64+.
            v_ext = v_exts[p % 2]
            vt_insts = []
            ps_vt = ps_vt_p.tile([113, 3, 128], BF, tag="vt", name="ps_vt")
            for ki in range(3):
                t = nc.tensor.transpose(
                    ps_vt[0:113, ki, :],
                    vpad[:, ki, 128 * p:128 * p + 113],
                    ident[:],
                    tile_position=(0, 0))
                for prev in qk_b1:
                    tile.add_dep_helper(t.ins, prev,
                                        reason="qk-b1 before vt")
                vt_insts.append(t.ins)
            for w in range(2):
                nc.scalar.copy(
                    v_ext[64 * w:64 * w + N, :, 0:HD],
                    ps_vt[64 * w:64 * w + N, :, :].rearrange(
                        "p a (b c) -> p (a b) c", c=HD))

            # QK batch 2: heads with h%4 in {2,3}, after the barrier
            for w in range(2):
                wc0 = pc0 + w * N
                for h in range(12):
                    if h % 4 < 2:
                        continue
                    mm = nc.tensor.matmul(
                        ps_s[64 * w:64 * w + N, h % 2,
                             scol(h):scol(h) + 49],
                        qkvT[32 * (h % 4):32 * (h % 4) + 32, 3 + h // 4,
                             wc0:wc0 + N],
                        qkvT[32 * (h % 4):32 * (h % 4) + 32, h // 4,
                             wc0:wc0 + N],
                        start=True, stop=True,
                        tile_position=(32 * (h % 4), 64 * w))
                    for vt in vt_insts:
                        tile.add_dep_helper(mm.ins, vt,
                                            reason="vt before qk-b2")

            # exp directly from PSUM; ps_s = QS^2/SCALE^-1... = QS^2 * S/scale
            e_sb = attn_p.tile([113, 2, 294], F32, tag="e", name="e_sb")
            nc.scalar.activation(e_sb[:], ps_s[:, :, 0:294], EXPF,
                                 scale=float(SCALE / (QS * QS)))
            # SBUF-only multiply on the otherwise-idle Pool engine
            ep = attn_p.tile([113, 2, 294], BF, tag="ep", name="ep")
            nc.gpsimd.tensor_tensor(ep[:], e_sb[:], expb[:],
                                    mybir.AluOpType.mult)

            # AV (+ rowsum in col 32 via ones column of v_ext);
            # psum bank = w so w0/w1 row-group sets never share a bank
            ps_o_t = ps_o_p.tile([113, 2, 512], F32, tag="o", name="ps_o")
            ps_o = [
                ps_o_t[64 * w:64 * w + N, w, 0:H * (HD + 1)].rearrange(
                    "p (h c) -> p h c", c=HD + 1)
                for w in range(2)
            ]
            for w in range(2):
                for h in range(12):
                    nc.tensor.matmul(
                        ps_o[w][:, h, :],
                        ep[64 * w:64 * w + N, h % 2,
                           scol(h):scol(h) + 49],
                        v_ext[64 * w:64 * w + N, h, :],
                        start=True, stop=True,
                        tile_position=(64 * w, 64 * w))

            r = attn_p.tile([113, H, 1], F32, tag="r", name="r")
            for w in range(2):
                nc.vector.reciprocal(r[64 * w:64 * w + N, :, :],
                                     ps_o[w][:, :, HD:HD + 1])
            o_sb = o_sbs[p % 2]
            for w in range(2):
                nc.vector.tensor_tensor(
                    o_sb[64 * w:64 * w + N, :, :],
                    ps_o[w][:, :, 0:HD],
                    r[64 * w:64 * w + N, :, :].to_broadcast([N, H, HD]),
                    mybir.AluOpType.mult)

        def emit_B_back(c, p):
            oT = oT_tiles[c]
            oTq = oTq_tiles[c]
            pc0 = p * 2 * N
            o_sb = o_sbs[p % 2]
            # O^T: transpose [113, 128] -> [128, 113], copy valid halves.
            # Allocated from the ps_lin ring (not ps_vt) so the next pair's
            # V transposes never wait on this tile's Act-copy consumers.
            ps_ot = ps_lin.tile([128, 3, 128], BF, tag="lin", name="ps_ot")
            for ki in range(3):
                nc.tensor.transpose(ps_ot[:, ki, 0:113],
                                    o_sb[:, 4 * ki:4 * ki + 4, :],
                                    ident[0:113, 0:113])
            for w in range(2):
                nc.scalar.copy(
                    oT[:, :, pc0 + w * N:pc0 + (w + 1) * N],
                    ps_ot[:, :, 64 * w:64 * w + N])
                nc.scalar.copy(
                    oTq[:, 0:3, pc0 + w * N:pc0 + (w + 1) * N],
                    ps_ot[:, :, 64 * w:64 * w + N])

        def emit_C(c):
            """proj LoRA-linear for chunk c; yields after pxd and each m-tile."""
            t0 = c * TCHUNK
            oT = oT_tiles.pop(c)
            oTq = oTq_tiles.pop(c)
            qkvT_tiles.pop(c, None)
            # (qkvT/vpad tiles release via pool rotation)
            ps_pxd = ps_lin.tile([128, 512], F32, tag="lin", name="ps_pxd")[0:R, 0:TCHUNK]
            nc.tensor.matmul(ps_pxd, pdownQ[:, 0:2, :], oTq[:, 0:2, :],
                             start=True, stop=False, perf_mode=DR)
            nc.tensor.matmul(ps_pxd, pdownQ[:, 2:4, :], oTq[:, 2:4, :],
                             start=False, stop=True, perf_mode=DR)
            pxd = sb_p.tile([R, 2, TCHUNK], FP8, tag="xd", name="pxd")
            nc.gpsimd.memset(pxd[:, 1, :], 0.0)
            # ps_pxd = LIFT*QS * xd2; Identity allows the per-partition bias
            nc.scalar.activation(pxd[:, 0, :], ps_pxd[:], IDENT,
                                 bias=cdQ[:], scale=float(1.0 / (LIFT * QS)))
            yield

            out_sb = out_p.tile([128, 3, TCHUNK], F32, name="out_sb")
            for mi in range(3):
                ps_g2 = ps_lin.tile([128, 512], F32, tag="lin", name="ps_g2")[:, 0:TCHUNK]
                nc.tensor.matmul(ps_g2, pgateQ[:, 0:2, 128 * mi:128 * mi + 128],
                                 oTq[:, 0:2, :], start=True, stop=False,
                                 perf_mode=DR)
                nc.tensor.matmul(ps_g2, pgateQ[:, 2:4, 128 * mi:128 * mi + 128],
                                 oTq[:, 2:4, :], start=False, stop=True,
                                 perf_mode=DR)
                t2 = sb_p.tile([128, TCHUNK], BF, tag="g", name="t2")
                # ps_g2 = LIFT*QS*z2 -> tanh((z2+cg)/2)
                nc.scalar.activation(t2[:], ps_g2[:], TANH,
                                     bias=cgQ[:, mi:mi + 1],
                                     scale=float(0.5 / (LIFT * QS)))

                ps_l2 = ps_lin.tile([128, 512], F32, tag="lin", name="ps_l2")[:, 0:TCHUNK]
                nc.tensor.matmul(ps_l2, pupQ[:, :, 128 * mi:128 * mi + 128],
                                 pxd[:], start=True, stop=True, perf_mode=DR)

                ps_m2 = ps_lin.tile([128, 512], F32, tag="lin", name="ps_m2")[:, 0:TCHUNK]
                for ki in range(3):
                    nc.tensor.matmul(ps_m2, pwT[:, ki, 128 * mi:128 * mi + 128],
                                     oT[:, ki, :],
                                     start=(ki == 0), stop=(ki == 2))
                gl2 = sb_p.tile([128, TCHUNK], F32, tag="gl", name="gl2")
                nc.vector.scalar_tensor_tensor(gl2[:], t2[:], 1.0, ps_l2[:],
                                               ADD, MULT)
                nc.vector.scalar_tensor_tensor(out_sb[:, mi, :], ps_m2[:],
                                               biasQ[:, 9 + mi:10 + mi],
                                               gl2[:], ADD, ADD)
                yield

            nc.sync.dma_start(out=outT_d[:, :, t0:t0 + TCHUNK], in_=out_sb)


        # clean schedule: for each c, emit A(c) m-tiles interleaved with
        # B pairs of c-1, then C(c-1) m-tiles interleaved into A(c) tail.
        prev_b = None  # chunk index whose B/C still need emission
        for c in range(NCHUNK):
            a_gen = emit_A(c)
            plan = []
            if prev_b is not None:
                for p in range(NPAIR):
                    plan.append(("Bf", (prev_b, p)))
                    plan.append(("Bb", (prev_b, p)))
                plan.append(("C", prev_b))
            # a_gen yields 10 times (xd + 9 m-tiles)
            c_sub = None
            for step in range(10):
                try:
                    next(a_gen)
                except StopIteration:
                    break
                if step == 0:
                    continue
                if plan:
                    kind, arg = plan[0]
                    if kind == "Bf":
                        emit_B_front(*arg)
                        plan.pop(0)
                    elif kind == "Bb":
                        emit_B_back(*arg)
                        plan.pop(0)
                    else:
                        if c_sub is None:
                            c_sub = emit_C(arg)
                        try:
                            next(c_sub)
                        except StopIteration:
                            c_sub = None
                            plan.pop(0)
            # finish any leftover B/C of prev chunk
            while plan:
                kind, arg = plan.pop(0)
                if kind == "Bf":
                    emit_B_front(*arg)
                elif kind == "Bb":
                    emit_B_back(*arg)
                else:
                    if c_sub is None:
                        c_sub = emit_C(arg)
                    for _ in c_sub:
                        pass
                    c_sub = None
            if c_sub is not None:
                for _ in c_sub:
                    pass
                c_sub = None
            prev_b = c
        # trailing B + C of the last chunk
        for p in range(NPAIR):
            emit_B_front(prev_b, p)
            emit_B_back(prev_b, p)
        for _ in emit_C(prev_b):
            pass

        for pool in reversed((consts, xt_p, qkvT_p, sb_p, attn_p, oT_p, out_p,
                              ps_lin, ps_s_p, ps_o_p, ps_vt_p)):
            pool.release()

    nc.compile()
    return nc


def _get_nc(has_mask: bool):
    if has_mask not in _COMPILED:
        _COMPILED[has_mask] = _build(has_mask)
    return _COMPILED[has_mask]


def _arr_lhsT(w_t, kparts):
    """[K, M] -> [128, K//128, M] partition-tiled lhsT layout."""
    K, M = w_t.shape
    return np.ascontiguousarray(
        w_t.reshape(kparts, 128, M).transpose(1, 0, 2))


def _pack_dr(w_t):
    """[K<=384, M] f32 -> [128, 4, M] DoubleRow-planes (plane 3 zeros)."""
    K, M = w_t.shape
    out = np.zeros((128, 4, M), np.float32)
    for plane in range(4):
        lo = plane * 128
        hi = min(lo + 128, K)
        if lo < K:
            out[0:hi - lo, plane] = w_t[lo:hi]
    return out


def _q8(a):
    return np.asarray(a, np.float32).astype(NPFP8)


def _prep_inputs(x, mask, qkv_w, qkv_b, qkv_down, qkv_up, qkv_gate, qkv_res,
                 proj_w, proj_b, proj_down, proj_up, proj_gate, proj_res,
                 bias_table, rel_index):
    x = np.asarray(x, np.float32)
    mask = np.asarray(mask, np.float32)
    has_mask = bool(np.any(mask))

    qkv_b = np.asarray(qkv_b, np.float32)
    proj_b = np.asarray(proj_b, np.float32)
    w_eff = (np.asarray(qkv_w, np.float32)
             + np.asarray(qkv_res, np.float32))        # [1152, 384]
    pw_eff = (np.asarray(proj_w, np.float32)
              + np.asarray(proj_res, np.float32))      # [384, 384]
    pg = np.asarray(proj_gate, np.float32)
    pd = np.asarray(proj_down, np.float32)

    bv = qkv_b[2 * D:3 * D]                            # v bias, folded below
    bp_eff = proj_b + bv @ pw_eff.T                    # main-path fold
    cg = bv @ pg.T                                     # gate-path fold
    cd = bv @ pd.T                                     # lora-down fold

    biasQ = np.zeros((128, 12), np.float32)
    for mi in range(3):                                # q bias only (k/v: none)
        biasQ[:, mi] = QS * qkv_b[128 * mi:128 * mi + 128]
    for mi in range(3):
        biasQ[:, 9 + mi] = QS * bp_eff[128 * mi:128 * mi + 128]

    common = {
        "wqkvT": _arr_lhsT((QS * w_eff).T, 3).astype(BF16),
        "gateQ": _q8(_pack_dr(LIFT * np.asarray(qkv_gate, np.float32).T)),
        "downQ": _q8(_pack_dr(LIFT * np.asarray(qkv_down, np.float32).T)),
        "pwT": _arr_lhsT(pw_eff.T, 3).astype(BF16),
        "pgateQ": _q8(_pack_dr(LIFT * pg.T)),
        "pdownQ": _q8(_pack_dr(LIFT * pd.T)),
        "biasQ": biasQ,
        "cgQ": np.ascontiguousarray(
            0.5 * cg.reshape(3, 128).T.astype(np.float32)),
        "cdQ": cd.reshape(R, 1).astype(np.float32),
        "ident": np.eye(128, dtype=BF16),
    }
    # lora-up weights carry the tanh 1/2: 0.5*SCALING = 1.0
    upq = np.zeros((R, 2, 3 * D), np.float32)
    upq[:, 0, :] = LIFT * (0.5 * SCALING) * np.asarray(qkv_up, np.float32).T
    common["upQ"] = _q8(upq)
    pupq = np.zeros((R, 2, D), np.float32)
    pupq[:, 0, :] = LIFT * (0.5 * SCALING) * np.asarray(proj_up, np.float32).T
    common["pupQ"] = _q8(pupq)

    # exp(relative-position bias + mask) in S^T layout [m, bank*294+scol(h)+n]
    bt = np.asarray(bias_table, np.float32)
    ri = np.asarray(rel_index).astype(np.int64)
    b_nmh = bt[ri]                                # [n, m, H]
    hperm = np.argsort(
        [(h % 2) * 6 + 3 * ((h % 4) // 2) + h // 4 for h in range(H)])

    def _eT(lg):                                  # lg [n, m, H] -> [m, 588]
        e = np.exp(lg).transpose(1, 2, 0)         # [m, H, n]
        return e[:, hperm, :].reshape(N, 588)

    if has_mask:
        expb = np.zeros((32, 113, 588), np.float32)
        for pm in range(32):
            for w01 in range(2):
                lg = b_nmh + mask[2 * pm + w01][:, :, None]   # [n, m, H]
                expb[pm, 64 * w01:64 * w01 + N] = _eT(lg)
    else:
        expb = np.zeros((1, 113, 588), np.float32)
        eT = _eT(b_nmh)
        expb[0, 0:N] = eT
        expb[0, 64:64 + N] = eT
    common["expb"] = expb

    in_maps = []
    for core in range(NCORES):
        tok = np.ascontiguousarray(
            x[core * WPC:(core + 1) * WPC].reshape(TPC, D))
        xt = np.ascontiguousarray(
            tok.reshape(TPC, 3, 128).transpose(2, 1, 0)).astype(BF16)
        xq = np.zeros((128, 4, TPC), NPFP8)
        xq[:, 0:3, :] = xt.astype(np.float32).astype(NPFP8)
        m = dict(common)
        m["xt"] = xt
        m["xq"] = xq
        in_maps.append(m)
    return has_mask, in_maps


def _gather(res):
    outs = []
    for core in range(NCORES):
        ot = res.results[core]["outT"]            # [128, 3, TPC] f32 (x QS)
        out = np.ascontiguousarray(ot.transpose(2, 1, 0)).reshape(TPC, D)
        outs.append(out)
    full = np.concatenate(outs, axis=0).reshape(B_, N, D)
    return (full * (1.0 / QS)).astype(np.float32)


def kernel(**inputs):
    has_mask, in_maps = _prep_inputs(**inputs)
    nc = _get_nc(has_mask)
    res = run_bass_kernel_spmd(nc, in_maps, list(range(NCORES)))
    return _gather(res)


def run_traced(**inputs):
    """Like kernel() but with NTFF profiling; returns (out, BassKernelResults)."""
    sys.path.insert(0, "/root/problem")
    import profhook
    profhook.install()
    has_mask, in_maps = _prep_inputs(**inputs)
    nc = _get_nc(has_mask)
    res = run_bass_kernel_spmd(nc, in_maps, list(range(NCORES)), trace=True)
    return _gather(res), res
